# revision 3
# baseline (speedup 1.0000x reference)
"""Bidirectional LSTM (B=32, T=512, D=H=512) on 8 Trainium2 NeuronCores.

Strategy:
  - One SPMD program on all 8 cores. Core 0 runs the forward direction,
    core 1 runs the backward direction (same instruction stream, fed
    time-reversed x and the bw weights). Other cores run the same program
    on replicated data; their outputs are ignored.
  - xp = x @ Wx is computed by the same core: the first T-chunk as a
    prologue, later chunks interleaved into the recurrence steps
    (quarter-m-tile quanta) so the PE stays busy (and HAM-warm) during
    each step's ACT/DVE tail.
  - Per step, PSUM accumulates z = I33 @ [xp_t; b] + sum_k hT_k.T @ Wh_k
    (fp32r matmuls, batch=32-wide stationary). Gate columns are
    host-permuted into bank order [f | g_lo i_lo | g_hi i_hi | o].
    The tail runs at half-hidden granularity; c_new is written back into
    the freed half of the g/i PSUM bank so tanh(c) reads PSUM at 1x rate.
    h is materialized directly in transposed (stationary) layout by
    PE-transposing sigmoid(o) and tanh(c) and multiplying on DVE.
  - Output is written as [T, H, B] per direction and reassembled on host.
"""

import os
import sys
import numpy as np

for _p in ("/opt/trn_rl_repo", "/root/.axon_site/_ro/trn_rl_repo"):
    if os.path.isdir(_p) and _p not in sys.path:
        sys.path.insert(0, _p)

B, T, D, H = 32, 512, 512, 512
G = 4 * H
N_CORES = 8

_PROG_CACHE = {}


def _build_program(t_steps=T, reps=1):
    from contextlib import ExitStack
    import concourse.bacc as bacc
    import concourse.tile as tile
    import concourse.mybir as mybir
    from concourse import masks

    f32 = mybir.dt.float32
    f32r = mybir.dt.float32r
    AF = mybir.ActivationFunctionType

    nc = bacc.Bacc("TRN2", target_bir_lowering=False, debug=False,
                   num_devices=N_CORES)

    x_t = nc.dram_tensor("x", [B, t_steps, D], f32, kind="ExternalInput")
    Wx_t = nc.dram_tensor("Wx", [D, G], f32r, kind="ExternalInput")
    Wh_t = nc.dram_tensor("Wh", [H, G], f32r, kind="ExternalInput")
    bv_t = nc.dram_tensor("bv", [G], f32r, kind="ExternalInput")
    idb_t = nc.dram_tensor("idb", [33, 32], f32r, kind="ExternalInput")
    out_t = nc.dram_tensor("out_h", [t_steps, H, B], f32r, kind="ExternalOutput")

    TCH = min(128, t_steps)
    TC = t_steps // TCH

    with tile.TileContext(nc) as tc, ExitStack() as ctx:
        wpool = ctx.enter_context(tc.tile_pool(name="w", bufs=1))
        hpool = ctx.enter_context(tc.tile_pool(name="hst", bufs=2))
        tpool = ctx.enter_context(tc.tile_pool(name="tmp", bufs=3))
        xpool = ctx.enter_context(tc.tile_pool(name="xin", bufs=2))
        ppool = ctx.enter_context(tc.tile_pool(name="ps", bufs=1, space="PSUM"))
        tppool = ctx.enter_context(tc.tile_pool(name="tps", bufs=2, space="PSUM"))
        p1pool = ctx.enter_context(tc.tile_pool(name="p1s", bufs=1, space="PSUM"))
        cpool = ctx.enter_context(tc.tile_pool(name="cs", bufs=1, space="PSUM"))
        dpool = ctx.enter_context(tc.tile_pool(name="dram", bufs=1, space="DRAM"))

        for _rep in range(reps):
            ident = wpool.tile([128, 128], f32)
            masks.make_identity(nc, ident[:])

            idb_sb = wpool.tile([33, 32], f32r)
            nc.sync.dma_start(idb_sb[:], idb_t.ap())

            Wx_sb = wpool.tile([128, 4, G], f32r, tag="Wbig")
            for k in range(4):
                nc.sync.dma_start(Wx_sb[:, k, :], Wx_t.ap()[k * 128:(k + 1) * 128, :])
            Wh_sb = wpool.tile([128, 4, G], f32r, tag="Wbig2")
            for k in range(4):
                nc.sync.dma_start(Wh_sb[:, k, :], Wh_t.ap()[k * 128:(k + 1) * 128, :])

            xp_dram = dpool.tile([t_steps, B, G], f32r)

            # ---- phase-1 quarter-m-tile quantum emitter -------------------
            p1_state = {}

            def emit_p1_quarter(tcki, b, n):
                tsl = slice(tcki * TCH, (tcki + 1) * TCH)
                if n == 0:
                    xt = xpool.tile([TCH, D], f32, tag="xt")
                    nc.sync.dma_start(xt[:], x_t.ap()[b, tsl, :])
                    xT_ps = p1pool.tile([128, 4, TCH], f32, tag="p1")
                    for k in range(4):
                        nc.tensor.transpose(xT_ps[:, k, :],
                                            xt[:, k * 128:(k + 1) * 128],
                                            ident[0:TCH, 0:TCH])
                    xT_sb = xpool.tile([128, 4, TCH], f32r, tag="xT")
                    nc.vector.tensor_copy(xT_sb[:], xT_ps[:])
                    zx = xpool.tile([TCH, G], f32r, tag="zx")
                    p1_state["xT"] = xT_sb
                    p1_state["zx"] = zx
                xT_sb = p1_state["xT"]
                zx = p1_state["zx"]
                zq = p1pool.tile([TCH, 512], f32, tag="p1")
                for k in range(4):
                    nc.tensor.matmul(zq[:], xT_sb[:, k, :],
                                     Wx_sb[:, k, n * 512:(n + 1) * 512],
                                     start=(k == 0), stop=(k == 3))
                nc.vector.tensor_copy(zx[:, n * 512:(n + 1) * 512], zq[:])
                if n == 3:
                    nc.sync.dma_start(xp_dram[tsl, b, :], zx[:])

            # quarter schedule: chunk 0 in the prologue; chunk c>0 at
            # 2 quarters/step over steps [TCH*(c-1), TCH*(c-1)+64).
            step_quanta = {}
            for c in range(1, TC):
                for q in range(4 * B):
                    st = TCH * (c - 1) + q // 2
                    step_quanta.setdefault(st, []).append((c, q // 4, q % 4))

            for b in range(B):
                for n in range(4):
                    emit_p1_quarter(0, b, n)

            # ---------------- recurrence ------------------------------------
            # bank layout: 0 = f | 1 = [g_lo, i_lo] | 2 = [g_hi, i_hi] | 3 = o
            RING = 6
            xr = wpool.tile([33, RING, G], f32r, tag="xr")
            for s in range(RING):
                nc.sync.dma_start(xr[32:33, s, :], bv_t.ap()[None, :])

            zf = wpool.tile([128, 4, B], f32, tag="zf")
            nc.vector.memset(zf[:], 0.0)
            hT = hpool.tile([128, 4, B], f32r, tag="hT")
            nc.vector.tensor_copy(hT[:], zf[:])
            # persistent cell state, lives in one PSUM bank (in-place update;
            # DVE is in-order so the read-then-overwrite within a step is safe)
            c_ps = cpool.tile([B, H], f32, tag="cps")
            nc.vector.memset(c_ps[:], 0.0)

            HH = H // 2
            for t in range(t_steps):
                s = t % RING
                nc.sync.dma_start(xr[0:32, s, :], xp_dram[t, :, :])

                zp = ppool.tile([B, 4, 512], f32, tag="z")
                for n in range(4):
                    nsl = slice(n * 512, (n + 1) * 512)
                    nc.tensor.matmul(zp[:, n, :], idb_sb[:], xr[:, s, nsl],
                                     start=True, stop=False)
                    for k in range(4):
                        nc.tensor.matmul(zp[:, n, :], hT[:, k, :],
                                         Wh_sb[:, k, nsl],
                                         start=False, stop=(k == 3))

                # --- tail ---
                sf = tpool.tile([B, H], f32, tag="sf")
                nc.scalar.activation(sf[:], zp[:, 0, :], AF.Sigmoid)
                t2 = tpool.tile([B, H], f32, tag="t2")
                nc.vector.tensor_mul(t2[:], sf[:], c_ps[:])

                tcl = tpool.tile([B, H], f32, tag="tc")
                for j in (0, 1):
                    hsl = slice(j * HH, (j + 1) * HH)
                    tg = tpool.tile([B, HH], f32, tag=f"tg{j}")
                    nc.scalar.activation(tg[:], zp[:, 1 + j, 0:HH], AF.Tanh)
                    si = tpool.tile([B, HH], f32, tag=f"si{j}")
                    nc.scalar.activation(si[:], zp[:, 1 + j, HH:512], AF.Sigmoid)
                    t1 = tpool.tile([B, HH], f32, tag=f"t1{j}")
                    nc.vector.tensor_mul(t1[:], si[:], tg[:])
                    nc.vector.tensor_add(c_ps[:, hsl], t1[:], t2[:, hsl])
                    nc.scalar.activation(tcl[:, hsl], c_ps[:, hsl], AF.Tanh)

                so = tpool.tile([B, H], f32, tag="so")
                nc.scalar.activation(so[:], zp[:, 3, :], AF.Sigmoid)

                soT = tppool.tile([128, 4, B], f32, tag="tp")
                tcT = tppool.tile([128, 4, B], f32, tag="tp")
                soT_sb = tpool.tile([128, 4, B], f32, tag="soTs")
                hT_new = hpool.tile([128, 4, B], f32r, tag="hT")
                for k in range(4):
                    nc.tensor.transpose(soT[:, k, :], so[:, k * 128:(k + 1) * 128],
                                        ident[0:B, 0:B])
                    nc.tensor.transpose(tcT[:, k, :], tcl[:, k * 128:(k + 1) * 128],
                                        ident[0:B, 0:B])
                for j in (0, 1):
                    ksl = slice(2 * j, 2 * j + 2)
                    nc.vector.tensor_copy(soT_sb[:, ksl, :], soT[:, ksl, :])
                    nc.vector.tensor_mul(hT_new[:, ksl, :], tcT[:, ksl, :],
                                         soT_sb[:, ksl, :])

                nc.sync.dma_start(out_t.ap()[t].rearrange("(k p) b -> p k b", p=128),
                                  hT_new[:])

                for (c, bq, nq) in step_quanta.get(t, ()):
                    emit_p1_quarter(c, bq, nq)

                hT = hT_new

    nc.compile()
    return nc


def _get_program(t_steps=T, reps=1):
    key = (t_steps, reps)
    if key not in _PROG_CACHE:
        _PROG_CACHE[key] = _build_program(t_steps, reps)
    return _PROG_CACHE[key]


def _permute_gates(W, b):
    # reference gate order [i, f, o, g] (each H wide) -> kernel bank order
    # [f | g_lo, i_lo | g_hi, i_hi | o]
    i_, f_, o_, g_ = (W[:, k * H:(k + 1) * H] for k in range(4))
    ib, fb, ob, gb = (b[k * H:(k + 1) * H] for k in range(4))
    HH = H // 2
    Wg = np.concatenate([f_, g_[:, :HH], i_[:, :HH], g_[:, HH:], i_[:, HH:], o_], axis=1)
    bg = np.concatenate([fb, gb[:HH], ib[:HH], gb[HH:], ib[HH:], ob])
    return np.ascontiguousarray(Wg), np.ascontiguousarray(bg)


LAST_EXEC_NS = None
LAST_TRACE = None


def kernel(x, W_fw, b_fw, W_bw, b_bw, t_steps=None, trace=False):
    global LAST_EXEC_NS, LAST_TRACE
    from concourse.bass_utils import run_bass_kernel_spmd

    x = np.asarray(x, dtype=np.float32)
    ts = t_steps or x.shape[1]
    nc = _get_program(ts)

    idb = np.zeros((33, 32), np.float32)
    idb[:32, :32] = np.eye(32, dtype=np.float32)
    idb[32, :] = 1.0

    Wf, bf = _permute_gates(np.asarray(W_fw, np.float32), np.asarray(b_fw, np.float32))
    Wb, bb = _permute_gates(np.asarray(W_bw, np.float32), np.asarray(b_bw, np.float32))

    x_rev = np.ascontiguousarray(x[:, ::-1])

    core0 = {"x": x, "Wx": np.ascontiguousarray(Wf[:D]),
             "Wh": np.ascontiguousarray(Wf[D:]), "bv": bf, "idb": idb}
    core1 = {"x": x_rev, "Wx": np.ascontiguousarray(Wb[:D]),
             "Wh": np.ascontiguousarray(Wb[D:]), "bv": bb, "idb": idb}
    in_maps = [core0, core1] + [core0] * (N_CORES - 2)

    if trace:
        res = run_bass_kernel_spmd(nc, in_maps, list(range(N_CORES)),
                                   trace=True, trace_cores=[0])
        LAST_EXEC_NS = res.exec_time_ns
        if res.instructions_and_trace is not None:
            LAST_TRACE = res.instructions_and_trace[1]
    else:
        res = run_bass_kernel_spmd(nc, in_maps, list(range(N_CORES)))

    h_fw = res.results[0]["out_h"].transpose(2, 0, 1)          # [B, T, H]
    h_bw = res.results[1]["out_h"][::-1].transpose(2, 0, 1)
    return np.ascontiguousarray(
        np.concatenate([h_fw, h_bw], axis=-1).astype(np.float32))

